# revision 58
# baseline (speedup 1.0000x reference)
"""CALoraLinear kernel for 8 TRN2 NeuronCores (Bass/Tile, SPMD).

Math (derived from the reference):
  orig = x @ W.T + bias
  top2 classes c1,c2 per row from pseudo_index[b, :64]
  g_j = <lora_A[c_j], x[b]>          (only rows 0..63 of lora_A are reachable)
  lora_out[b,o] = 16 * sum_c mask[b,c] * G[b,c] * lora_B[o,c]
  out = orig + lora_out + bias       (bias added twice)

Sharding: column-shard W across the 8 cores (each core owns 512 output
columns, full batch); x / lora_A / pseudo_index replicated. Host
concatenates the per-core [512, 512] blocks along the output axis.

Perf structure (v2):
  - fp16 operand streaming (PE upconverts 2B floats to FP22 internally).
  - One DMA per k-tile (32 chunks of ~278KB) alternating the two HWDGE
    rings, so the first chunk lands ~2x sooner than 2-k chunks.
  - PE warm-up: a run of dummy matmuls on a zeroed tile keeps the PE busy
    through the initial DMA window so the HAM un-throttles (1.2->2.4GHz)
    before the real matmul stream starts.
  - Early class-major mask (is_ge of psT vs bcast top-2 threshold).
  - Short tail: single DVE ADD of the two packed G psum halves + fp16
    mask-multiply, fp16 tail matmuls (lora_B*16 + bias row pre-packed
    fp16), fp16 output DMA (host upcasts).
"""

import os
import sys

for _p in ("/opt/trn_rl_repo",):
    if _p not in sys.path:
        sys.path.insert(0, _p)

import numpy as np

import concourse.bass as bass
import concourse.bacc as bacc
import concourse.mybir as mybir
from concourse.tile import TileContext, add_dep_helper
from concourse.bass_utils import run_bass_kernel_spmd


def _ensure_ntff_hook_module():
    """run_bass_kernel_spmd(trace=True) imports antenv.axon_hooks, which the
    agent image's antenv package lacks. Provide it (and register the real
    ctypes NTFF hook when available) so a tracing caller doesn't crash."""
    import types

    try:
        import antenv
    except ImportError:
        return
    if getattr(antenv, "axon_hooks", None) is not None:
        return
    mod = types.ModuleType("antenv.axon_hooks")
    state = {"hook": None}
    mod.set_axon_ntff_profile_hook = lambda h: state.__setitem__("hook", h)
    mod.get_axon_ntff_profile_hook = lambda: state["hook"]
    sys.modules["antenv.axon_hooks"] = mod
    antenv.axon_hooks = mod
    try:
        from trn_agent_boot.trn_boot import _ntff_profile_via_ctypes

        mod.set_axon_ntff_profile_hook(
            _ntff_profile_via_ctypes("/opt/axon/libaxon_pjrt.so")
        )
    except Exception:
        pass


_ensure_ntff_hook_module()

B, IN, OUT = 512, 4096, 4096
NUM_CLASS, RANK = 64, 8
NCORES = 8
OUT_L = OUT // NCORES  # 512
P = 128
KT = IN // P           # 32 k-tiles, one DMA chunk each
BT = B // P            # 4 batch tiles

# column layout of one k chunk: [a: 64][x: B][w: OUT_L]
# The G matmul needs a [128k, 128m] stationary (mixed-shape transitions
# measured ~+120ns per matmul), but only 64 real lora_A columns exist; its
# lhsT overreads 64 columns into the adjacent same-chunk x data, producing
# garbage in gt rows 64:128 which nothing reads. This avoids shipping a
# zero pad over DMA (~1MB across the 32 chunks).
AOFF = 0
XOFF = NUM_CLASS
WOFF = XOFF + B
W1 = WOFF + OUT_L  # 1088

# pp layout: [ps: BT*64][psT: B (rows 0:64)]
PSOFF = 0
PTOFF = BT * NUM_CLASS
PPW = PTOFF + B

F32 = mybir.dt.float32
F16 = mybir.dt.float16
X = mybir.AxisListType.X

# 10 dummy matmuls: ends ~11.6us at the 2.4GHz clock state (0.8us idle
# before chunk 0 lands at ~12.4us — harmless, HAM is already un-throttled)
# and ~12.4us at the degraded 2.0GHz state (cold dummies take 512ns there,
# clock-independent DMA still delivers chunk 0 at ~12.4us) — so it matches
# the DMA ramp across thermal states where 12 overshoots when hot.
N_WARM = int(os.environ.get("N_WARM", "10"))

# Chunk k is consumed in DMA-completion order, not index order: the SDMA
# engines round-robin across all queued transfers, and the observed
# completion order is c0, c2, c1, c4, c3, ... (sync-ring chunks lead their
# scalar-ring pair). Accumulation order over k is mathematically arbitrary.
K_ORDER = [0]
for _i in range(1, KT // 2):
    K_ORDER += [2 * _i, 2 * _i - 1]
K_ORDER += [KT - 1]

_cache = {}
# test.py reads this after a traced run for HW exec time
last_results = None


def _build():
    key = f"nc_v2_{N_WARM}"
    if key in _cache:
        return _cache[key]
    nc = bacc.Bacc(
        bass.get_trn_type() or "TRN2",
        target_bir_lowering=False,
        debug=False,
        num_devices=NCORES,
    )

    xw = nc.dram_tensor("xw", [KT, P, W1], F16, kind="ExternalInput")
    pp = nc.dram_tensor("pp", [P, PPW], F32, kind="ExternalInput")
    bs = nc.dram_tensor("bs", [NUM_CLASS + 1, OUT_L], F16, kind="ExternalInput")
    out = nc.dram_tensor("out", [B, OUT_L], F16, kind="ExternalOutput")

    with TileContext(nc) as tc:
        with (
            tc.tile_pool(name="xwp", bufs=1) as xwpool,
            tc.tile_pool(name="sml", bufs=1) as spool,
            tc.tile_pool(name="tl", bufs=1) as tpool,
            tc.tile_pool(name="op", bufs=1) as opool,
            tc.tile_pool(name="dr", bufs=1, space="DRAM") as dpool,
            tc.tile_pool(name="ps", bufs=1, space="PSUM") as ppool,
        ):
            # ---- PE warm-up: dummy matmuls on a zeroed tile. These have no
            # input dependency, so they run during the initial DMA window and
            # flip the HAM clock gate to 8/8 before the real stream starts.
            wtile = spool.tile([P, 640], F16)
            nc.gpsimd.memset(wtile, 0.0)
            warm_ps = ppool.tile([P, 512], F32, tag="warm", name="warm_ps")
            last_warm = None
            for i in range(N_WARM):
                last_warm = nc.tensor.matmul(
                    warm_ps,
                    lhsT=wtile[:, :P],
                    rhs=wtile[:, P:640],
                    start=True,
                    stop=True,
                )

            # ---- PSUM accumulators ----
            mps = [
                ppool.tile([P, OUT_L], F32, tag=f"main{bt}", name=f"main{bt}")
                for bt in range(BT)
            ]
            # G accumulates unpacked: one accumulation group over all 32
            # k-tiles, so the tail needs only a single mask-multiply. Rows
            # 64:128 are the zero-pad product (lhsT pad columns) and unused.
            gt_ps = ppool.tile([P, B], F32, tag="gt", name="gt_ps")

            # ---- small inputs ----
            pp_sb = spool.tile([P, PPW], F32)
            bs_sb = spool.tile([NUM_CLASS + 1, OUT_L], F16)
            ps_sb = pp_sb[:, PSOFF : PSOFF + BT * NUM_CLASS]
            psT_sb = pp_sb[:NUM_CLASS, PTOFF : PTOFF + B]

            # ---- all chunk DMAs issue first, one per k-tile, alternating
            # rings. The HBM ramp is slow and the SDMA engines round-robin
            # across every queued transfer, so a deep initial queue makes the
            # early chunks all complete late and nearly simultaneously.
            # Ramp control: a register "probe" read of an earlier chunk's
            # tile carries that chunk's DMA-completion wait on the issuing
            # engine, holding back the next issue and keeping the in-flight
            # depth shallow (near in-order completion) until the stream is
            # up; the HWDGE sem-lane rotation (depth 4/ring) takes over after.
            chunk_dmas = {}
            xwcs = [
                xwpool.tile([P, W1], F16, tag=f"xwc{k}", name=f"xwc{k}")
                for k in range(KT)
            ]
            # first chunk split across both rings
            nc.sync.dma_start(out=xwcs[0][:, :WOFF], in_=xw[0][:, :WOFF])
            chunk_dmas[0] = nc.scalar.dma_start(
                out=xwcs[0][:, WOFF:], in_=xw[0][:, WOFF:]
            )
            for k in range(1, KT):
                dma_eng = nc.sync if k % 2 == 0 else nc.scalar
                chunk_dmas[k] = dma_eng.dma_start(out=xwcs[k], in_=xw[k])
                if k == 1:
                    nc.scalar.dma_start(out=pp_sb, in_=pp[:, :])
                    nc.scalar.dma_start(out=bs_sb, in_=bs[:, :])

            # ---- top-2 threshold + class-major mask (DVE/SWDGE; runs
            # whenever its inputs land) ----
            m2col = spool.tile([P, BT], F32)
            for bt in range(BT):
                pt = ps_sb[:, bt * NUM_CLASS : (bt + 1) * NUM_CLASS]
                m1 = spool.tile([P, 1], F32, tag=f"m1_{bt}")
                nc.vector.reduce_max(out=m1, in_=pt, axis=X)
                negmask = spool.tile([P, NUM_CLASS], F32, tag=f"nm_{bt}")
                # (pt >= m1) * -1e30 -> additive mask killing the max
                nc.vector.tensor_scalar(
                    out=negmask,
                    in0=pt,
                    scalar1=m1,
                    scalar2=-1.0e30,
                    op0=mybir.AluOpType.is_ge,
                    op1=mybir.AluOpType.mult,
                )
                p2 = spool.tile([P, NUM_CLASS], F32, tag=f"p2_{bt}")
                nc.vector.tensor_tensor(
                    out=p2, in0=pt, in1=negmask, op=mybir.AluOpType.add
                )
                nc.vector.reduce_max(out=m2col[:, bt : bt + 1], in_=p2, axis=X)
            # partition->free shuffle via DRAM bounce + broadcast read, on
            # the SWDGE path; delayed past the chunk stream (it shares the
            # 16 SDMA engines and was observed starving the HWDGE chunk DMAs)
            m2d = dpool.tile([BT, P], F32)
            m2d_dma = nc.gpsimd.dma_start(
                out=m2d.rearrange("bt p -> p bt"), in_=m2col[:, :]
            )
            add_dep_helper(
                m2d_dma.ins,
                chunk_dmas[KT - 6].ins,
                reason="delay SWDGE bounce until the chunk stream tail",
            )
            thr_sb = spool.tile([NUM_CLASS, B], F32)
            nc.gpsimd.dma_start(
                out=thr_sb,
                in_=m2d.rearrange("bt p -> (bt p)")[None, :].broadcast_to(
                    [NUM_CLASS, B]
                ),
            )
            maskT = tpool.tile([NUM_CLASS, B], F16)
            nc.vector.tensor_tensor(
                out=maskT, in0=psT_sb, in1=thr_sb, op=mybir.AluOpType.is_ge
            )
            ht = tpool.tile([NUM_CLASS + 1, B], F16)
            # ones row (for the bias row of bs)
            nc.gpsimd.memset(ht[NUM_CLASS : NUM_CLASS + 1, :], 1.0)

            # ---- main matmul stream, in chunk-completion order ----
            first_mm = None
            for pos, k in enumerate(K_ORDER):
                xwc = xwcs[k]
                xk = xwc[:, XOFF:WOFF]
                wk = xwc[:, WOFF:W1]
                aT = xwc[:, 0:P]  # a + 64 same-chunk x cols (garbage M half)

                def _main(bt, xk=xk, wk=wk, pos=pos):
                    return nc.tensor.matmul(
                        mps[bt],
                        lhsT=xk[:, bt * P : (bt + 1) * P],
                        rhs=wk,
                        start=(pos == 0),
                        stop=False,
                    )

                def _g(aT=aT, xk=xk, pos=pos):
                    nc.tensor.matmul(
                        gt_ps,
                        lhsT=aT,
                        rhs=xk,
                        start=(pos == 0),
                        stop=(pos == KT - 1),
                    )

                if pos < KT - 2:
                    mm = _main(0)
                    if first_mm is None:
                        first_mm = mm
                        add_dep_helper(
                            first_mm.ins,
                            last_warm.ins,
                            reason="real MM stream starts after PE warm-up",
                        )
                    for bt in range(1, BT):
                        _main(bt)
                    _g()
                elif pos == KT - 2:
                    # both remaining G matmuls run here, ahead of this
                    # position's mains: the last chunk's data has landed by
                    # now (PE-bound regime), and issuing the final G five
                    # matmuls earlier lets the DVE mask-multiply chain finish
                    # before the tail matmuls need it (removes a reproducible
                    # ~0.7us stall on the last tail matmul)
                    _g()
                    k_last = K_ORDER[KT - 1]
                    xwl = xwcs[k_last]
                    nc.tensor.matmul(
                        gt_ps,
                        lhsT=xwl[:, 0:P],
                        rhs=xwl[:, XOFF:WOFF],
                        start=False,
                        stop=True,
                    )
                    for bt in range(BT):
                        sl = slice(bt * P, (bt + 1) * P)
                        nc.vector.tensor_tensor(
                            out=ht[0:NUM_CLASS, sl],
                            in0=gt_ps[0:NUM_CLASS, sl],
                            in1=maskT[:, sl],
                            op=mybir.AluOpType.mult,
                        )
                    for bt in range(BT):
                        _main(bt)
                else:
                    # last k: mains + tail matmuls interleaved (its G already
                    # ran in the previous position) so each output tile's
                    # cast/DMA starts as soon as its own column block is ready
                    for bt in range(BT):
                        sl = slice(bt * P, (bt + 1) * P)
                        _main(bt)
                        nc.tensor.matmul(
                            mps[bt],
                            lhsT=ht[:, sl],
                            rhs=bs_sb,
                            start=False,
                            stop=True,
                        )

            # ---- epilogue: PSUM -> SBUF (fp16) -> DRAM, pipelined per tile;
            # casts alternate DVE/ACT and the DMAs alternate the two rings so
            # consecutive tiles drain in parallel ----
            o_all = opool.tile([P, BT * OUT_L], F16)
            H = OUT_L // 2
            for bt in range(BT):
                if bt < BT - 1:
                    dst = o_all[:, bt * OUT_L : (bt + 1) * OUT_L]
                    if bt % 2 == 0:
                        nc.vector.tensor_copy(out=dst, in_=mps[bt])
                    else:
                        nc.scalar.copy(out=dst, in_=mps[bt])
                    dma_eng = nc.sync if bt % 2 == 0 else nc.scalar
                    dma_eng.dma_start(
                        out=out[bt * P : (bt + 1) * P, :],
                        in_=o_all[:, bt * OUT_L : (bt + 1) * OUT_L],
                    )
                else:
                    # last tile is on the critical path to the final drain:
                    # cast halves on both engines, DMA halves on both rings
                    for h in range(2):
                        dst = o_all[
                            :, bt * OUT_L + h * H : bt * OUT_L + (h + 1) * H
                        ]
                        src = mps[bt][:, h * H : (h + 1) * H]
                        if h == 0:
                            nc.vector.tensor_copy(out=dst, in_=src)
                        else:
                            nc.scalar.copy(out=dst, in_=src)
                        dma_eng = nc.sync if h == 0 else nc.scalar
                        dma_eng.dma_start(
                            out=out[bt * P : (bt + 1) * P, h * H : (h + 1) * H],
                            in_=dst,
                        )

    nc.finalize()
    _cache[key] = nc
    return nc


def _pack_inputs(x, pseudo_index, weight, bias, lora_A, lora_B):
    """Build the per-core xw buffers + replicated small inputs."""
    xT = np.ascontiguousarray(x.T)                   # [IN, B]
    aT = np.ascontiguousarray(lora_A[:NUM_CLASS].T)  # [IN, 64]
    x3 = xT.reshape(KT, P, B)
    a3 = aT.reshape(KT, P, NUM_CLASS)

    pp_base = np.zeros((P, PPW), dtype=np.float32)
    pp_base[:, PSOFF : PSOFF + BT * NUM_CLASS] = (
        pseudo_index.reshape(BT, P, NUM_CLASS)
        .transpose(1, 0, 2)
        .reshape(P, BT * NUM_CLASS)
    )
    pp_base[:NUM_CLASS, PTOFF : PTOFF + B] = pseudo_index.T

    in_maps = []
    for i in range(NCORES):
        o0 = i * OUT_L
        wTi = weight[o0 : o0 + OUT_L].T              # [IN, OUT_L] (view)
        w3 = wTi.reshape(KT, P, OUT_L)
        xwi = np.empty((KT, P, W1), dtype=np.float16)
        xwi[:, :, AOFF:XOFF] = a3
        xwi[:, :, XOFF:WOFF] = x3
        xwi[:, :, WOFF:W1] = w3
        bsi = np.empty((NUM_CLASS + 1, OUT_L), dtype=np.float16)
        bsi[:NUM_CLASS] = 16.0 * lora_B[o0 : o0 + OUT_L, :NUM_CLASS].T
        bsi[NUM_CLASS] = 2.0 * bias[o0 : o0 + OUT_L]
        in_maps.append({"xw": xwi, "pp": pp_base, "bs": bsi})
    return in_maps


def kernel(x, pseudo_index, weight, bias, lora_A, lora_B):
    global last_results
    x = np.ascontiguousarray(np.asarray(x, dtype=np.float32))
    pseudo_index = np.ascontiguousarray(np.asarray(pseudo_index, dtype=np.float32))
    weight = np.asarray(weight, dtype=np.float32)
    bias = np.asarray(bias, dtype=np.float32)
    lora_A = np.asarray(lora_A, dtype=np.float32)
    lora_B = np.asarray(lora_B, dtype=np.float32)

    nc = _build()
    in_maps = _pack_inputs(x, pseudo_index, weight, bias, lora_A, lora_B)
    res = run_bass_kernel_spmd(nc, in_maps, list(range(NCORES)))
    last_results = res
    return np.hstack(
        [res.results[i]["out"] for i in range(NCORES)]
    ).astype(np.float32)
